# revision 1
# baseline (speedup 1.0000x reference)
"""Multi-head self-attention (RoPE, eval-mode) Trainium2 Bass kernel.

Problem: B=2, T=2048, D=1024, H=16, d_head=64, fp32 I/O.

Sharding (8 cores): core c handles batch b=c//4 and the 4 heads
[4g, 4g+4) where g=c%4.  QKV/attention are head-local; the output
projection produces a per-core partial (contraction over this core's
256 head-dims) which the host sums across the 4 cores of each batch
and adds b_out.

Per-core design notes:
  - q,k are computed feature-major (d_head on partitions, T on free) so
    scores^T tiles come straight from matmuls; 2 heads stacked per
    128-partition tile, scores for both heads issued as row-packed
    (K=64) concurrent matmuls.
  - RoPE: rotate_half is a 32-partition block swap (SBUF->SBUF DMAs)
    with the sign folded into the host-provided sin table;
    q' = q*cos + rot(q)*sin_signed, with one mul on DVE, one on GpSimd.
    Rope work is emitted interleaved with remaining QKV matmuls so the
    PE never idles long enough for HAM to re-throttle the clock.
  - v is computed row-major [t, dv] and stored per head as [ones | v]
    128-wide stationary tiles, so each PV matmul yields the softmax
    denominators (partitions 0:64, replicated) and attn^T (64:128).
  - softmax skips max-subtraction (scores ~ N(0,1), exp safe in fp32)
    and normalizes after PV with the fast DVE reciprocal (base-0 only).
  - matmuls run as float32r (single-pass fp32 mode, full rate at
    moving dim >= 256).
  - attention uses tq=512 blocks: PSUM = 2 score tiles [128,1024]
    (both heads side by side, double buffered) + 2 PV accumulators
    [128,512] = 6 banks, one exp instr per tk tile.
"""

import numpy as np

B, T, D = 2, 2048, 1024
H = 16
DH = 64
NCORES = 8
P = 128

_CACHE = {}


def _rope_tables_np():
    theta = 1.0 / (10000.0 ** (np.arange(0, DH, 2, dtype=np.float32) / DH))
    angles = np.outer(np.arange(T, dtype=np.float32), theta)  # (T, 32)
    angles = np.concatenate([angles, angles], axis=-1)  # (T, DH)
    cos = np.cos(angles).astype(np.float32)
    sin = np.sin(angles).astype(np.float32)
    cosT = np.ascontiguousarray(cos.T)  # (64, T)
    sinT = np.ascontiguousarray(sin.T)
    sinT_signed = np.concatenate([-sinT[0:32], sinT[32:64]], axis=0)
    cos2 = np.tile(cosT, (2, 1))  # (128, T)
    sin2 = np.tile(sinT_signed, (2, 1))
    return cos2, sin2


def _build_module():
    import concourse.mybir as mybir
    import concourse.tile as tile
    from concourse import bacc

    f32 = mybir.dt.float32
    f32r = mybir.dt.float32r

    nc = bacc.Bacc("TRN2", target_bir_lowering=False, debug=False)
    xT = nc.dram_tensor("xT", [4, P, 8, 512], f32r, kind="ExternalInput")
    w_qk = nc.dram_tensor("w_qk", [P, 8, 512], f32r, kind="ExternalInput")
    w_v = nc.dram_tensor("w_v", [P, 8, 256], f32r, kind="ExternalInput")
    w_o = nc.dram_tensor("w_o", [P, 2, 1024], f32r, kind="ExternalInput")
    cos2 = nc.dram_tensor("cos2", [P, T], f32, kind="ExternalInput")
    sin2 = nc.dram_tensor("sin2", [P, T], f32, kind="ExternalInput")
    out = nc.dram_tensor("out", [T, D], f32, kind="ExternalOutput")

    Exp = mybir.ActivationFunctionType.Exp

    with tile.TileContext(nc) as tc:
        with tc.tile_pool(name="persist", bufs=1) as persist:
            wqk_sb = [
                persist.tile([P, 4, 512], f32r, tag=f"wqk{i}", name=f"wqk{i}")
                for i in range(2)
            ]
            wv_sb = persist.tile([P, 8, 256], f32r)
            # q_q[hp][qtr]: roped q, two heads stacked, per T-quarter.
            q_q = [
                [
                    persist.tile([P, 512], f32r, tag=f"q{hp}_{q}", name=f"q{hp}_{q}")
                    for q in range(4)
                ]
                for hp in range(2)
            ]
            # kpad[hp][h][qtr]: roped k per head, zero-padded to K=128 (head
            # 0 in rows 0:64, head 1 in rows 64:128, matching the stacked q
            # rhs).  All-K=128 matmuls keep the PE clock at full rate —
            # mixing row-grp K=64 mms with K=128 mms sticks HAM at 1.2 GHz.
            kpad = [
                [
                    [
                        persist.tile(
                            [P, 512], f32r, tag=f"kp{hp}{h}_{q}", name=f"kp{hp}{h}_{q}"
                        )
                        for q in range(4)
                    ]
                    for h in range(2)
                ]
                for hp in range(2)
            ]
            # per (tk-tile, head): [ones | v] stationary 128x128
            vaug = persist.tile([P, 16, 4, P], f32r)

            # Attention-phase SBUF pools open BEFORE the QKV pools so their
            # addresses never alias QKV workspace (an aliased exp output
            # would inherit a WAR dependency on the whole rope pipeline).
            with (
                tc.tile_pool(name="attnsb", bufs=1) as apool,
                tc.tile_pool(name="expp", bufs=3) as epool,
                tc.tile_pool(name="norm", bufs=1) as npool,
            ):
                attn_q = [
                    [
                        apool.tile(
                            [P, 512], f32r, tag=f"at{hp}_{b}", name=f"at{hp}_{b}"
                        )
                        for b in range(4)
                    ]
                    for hp in range(2)
                ]

                with (
                    tc.tile_pool(name="xt", bufs=2) as xpool,
                    tc.tile_pool(name="kst", bufs=1) as kpool,
                    tc.tile_pool(name="qkv_ps", bufs=2, space="PSUM") as qkps,
                    tc.tile_pool(name="rope", bufs=2) as rpool,
                ):
                    cos_sb = kpool.tile([P, T], f32)
                    sin_sb = kpool.tile([P, T], f32)
                    kstack = [
                        [
                            kpool.tile(
                                [P, 512], f32r, tag=f"ks{hp}_{q}", name=f"ks{hp}_{q}"
                            )
                            for q in range(4)
                        ]
                        for hp in range(2)
                    ]
                    for hp in range(2):
                        for q in range(4):
                            nc.vector.memset(
                                kpad[hp][0][q][64:128, :].bitcast(f32), 0.0
                            )
                            nc.vector.memset(
                                kpad[hp][1][q][0:64, :].bitcast(f32), 0.0
                            )

                    def fm_dst(cc, tq):
                        return (q_q if cc in (0, 2) else kstack)[cc // 2][tq]

                    def fm_chain(xt, tq, cc):
                        """One feature-major QKV chain (q or stacked k)."""
                        ps = qkps.tile([P, 512], f32, tag="fm", name="fmps")
                        for dc in range(8):
                            nc.tensor.matmul(
                                ps[:],
                                lhsT=wqk_sb[dc // 4][:, dc % 4, cc * P : (cc + 1) * P],
                                rhs=xt[dc // 4][:, dc % 4, :],
                                start=(dc == 0),
                                stop=(dc == 7),
                            )
                        nc.vector.tensor_copy(fm_dst(cc, tq)[:], ps[:])

                    def v_chain(xt, tq, t4):
                        psv = qkps.tile([P, 256], f32, tag="v", name="vps")
                        for dc in range(8):
                            nc.tensor.matmul(
                                psv[:],
                                lhsT=xt[dc // 4][:, dc % 4, t4 * P : (t4 + 1) * P],
                                rhs=wv_sb[:, dc, :],
                                start=(dc == 0),
                                stop=(dc == 7),
                            )
                        tki = tq * 4 + t4
                        nc.scalar.copy(
                            vaug[:, tki, :, 64:128],
                            psv.rearrange("p (h e) -> p h e", e=64),
                        )

                    def rope_q(cc, qtr):
                        """RoPE one T-quarter of one q/k tensor.  q is roped
                        in place; stacked k is roped into the per-head
                        zero-padded kpad tiles (two half-adds)."""
                        base = fm_dst(cc, qtr)
                        hs = slice(qtr * 512, (qtr + 1) * 512)
                        rot = rpool.tile([P, 512], f32, tag="rot", name="rot")
                        for blk in range(4):
                            s = (blk ^ 1) * 32
                            eng = nc.sync if blk % 2 == 0 else nc.gpsimd
                            eng.dma_start(
                                rot[blk * 32 : (blk + 1) * 32, :],
                                base[s : s + 32, :].bitcast(f32),
                            )
                        t1 = rpool.tile([P, 512], f32, tag="t1", name="t1")
                        nc.vector.tensor_mul(t1[:], base[:].bitcast(f32), cos_sb[:, hs])
                        nc.vector.tensor_mul(rot[:], rot[:], sin_sb[:, hs])
                        if cc in (0, 2):
                            nc.vector.tensor_add(base[:], t1[:], rot[:])
                        else:
                            hp = cc // 2
                            nc.vector.tensor_add(
                                kpad[hp][0][qtr][0:64, :], t1[0:64, :], rot[0:64, :]
                            )
                            nc.vector.tensor_add(
                                kpad[hp][1][qtr][64:128, :],
                                t1[64:128, :],
                                rot[64:128, :],
                            )

                    nc.scalar.dma_start(wqk_sb[0][:], w_qk[:, 0:4, :])
                    nc.gpsimd.dma_start(wqk_sb[1][:], w_qk[:, 4:8, :])
                    xts = []
                    for tq in range(4):
                        xtl = xpool.tile([P, 4, 512], f32r, tag="xtl", name="xtl")
                        xth = xpool.tile([P, 4, 512], f32r, tag="xth", name="xth")
                        nc.sync.dma_start(xtl[:], xT[tq, :, 0:4, :])
                        eng = nc.gpsimd if tq == 0 else nc.sync
                        eng.dma_start(xth[:], xT[tq, :, 4:8, :])
                        xts.append((xtl, xth))
                    nc.scalar.dma_start(wv_sb[:], w_v[:])
                    nc.vector.memset(vaug[:, :, :, 0:64].bitcast(f32), 1.0)
                    nc.scalar.dma_start(cos_sb[:], cos2[:])
                    nc.scalar.dma_start(sin_sb[:], sin2[:])

                    # Quarter-major; each quarter's ropes follow its chains
                    # so RoPE pipelines with QKV.
                    for tq in range(4):
                        for cc in (1, 0, 3, 2):
                            fm_chain(xts[tq], tq, cc)
                        for t4 in range(4):
                            v_chain(xts[tq], tq, t4)
                        for cc in (1, 0, 3, 2):
                            rope_q(cc, tq)

                # ---- attention + interleaved output projection ----------
                # Outproj for tq-block b is emitted right after attention
                # (hp1, b) so it hides under the remaining attention's ACT
                # time; PSUM: sc 4 + pv 2 + po 2 = 8 banks.
                with (
                    tc.tile_pool(name="wop", bufs=1) as wpool,
                    tc.tile_pool(name="ob", bufs=3) as opool,
                    tc.tile_pool(name="sc_ps", bufs=2, space="PSUM") as scps,
                    tc.tile_pool(name="pv_ps", bufs=1, space="PSUM") as pvps,
                    tc.tile_pool(name="po_ps", bufs=2, space="PSUM") as pops,
                ):
                    wo_sb = wpool.tile([P, 2, 1024], f32r)
                    nc.sync.dma_start(wo_sb[:], w_o[:])

                    def outproj_unit(b, tqc):
                        row = b * 4 + tqc
                        for d2 in range(2):
                            po = pops.tile([P, 512], f32, tag="po", name="po")
                            for hp in range(2):
                                nc.tensor.matmul(
                                    po[:],
                                    lhsT=attn_q[hp][b][:, tqc * P : (tqc + 1) * P],
                                    rhs=wo_sb[:, hp, d2 * 512 : (d2 + 1) * 512],
                                    start=(hp == 0),
                                    stop=(hp == 1),
                                )
                            ob = opool.tile([P, 512], f32, tag="ob", name="ob")
                            nc.vector.tensor_copy(ob[:], po[:])
                            seng = nc.sync if d2 == 0 else nc.gpsimd
                            seng.dma_start(
                                out[row * P : (row + 1) * P, d2 * 512 : (d2 + 1) * 512],
                                ob[:],
                            )

                    for hp in range(2):
                        for tq in range(4):  # tq blocks of 512
                            prev_b = tq - 1 if (hp == 1 and tq > 0) else None
                            pv = [
                                pvps.tile([P, 512], f32, tag=f"pv{h}", name=f"pv{h}")
                                for h in range(2)
                            ]
                            for tk in range(16):
                                if prev_b is not None and tk % 4 == 3:
                                    outproj_unit(prev_b, tk // 4)
                                sc = scps.tile([P, 1024], f32, tag="sc", name="sc")
                                ko = (tk % 4) * P
                                for h in range(2):
                                    nc.tensor.matmul(
                                        sc[:, h * 512 : (h + 1) * 512],
                                        lhsT=kpad[hp][h][tk // 4][:, ko : ko + P],
                                        rhs=q_q[hp][tq][:],
                                        start=True,
                                        stop=True,
                                    )
                                ex = epool.tile([P, 1024], f32r, tag="e", name="e")
                                nc.scalar.activation(ex[:], sc[:], Exp, scale=0.125)
                                for h in range(2):
                                    nc.tensor.matmul(
                                        pv[h][:],
                                        lhsT=vaug[:, tk, hp * 2 + h, :],
                                        rhs=ex[:, h * 512 : (h + 1) * 512],
                                        start=(tk == 0),
                                        stop=(tk == 15),
                                    )
                            for h in range(2):
                                rc = npool.tile([64, 512], f32, tag="rc", name="rc")
                                nc.vector.reciprocal_approx_fast(
                                    rc[:], pv[h][0:64, :]
                                )
                                hb = h * 64
                                nc.vector.tensor_mul(
                                    attn_q[hp][tq][hb : hb + 64, :],
                                    pv[h][64:128, :],
                                    rc[:],
                                )
                    for tqc in range(4):
                        outproj_unit(3, tqc)

    nc.compile()
    return nc


def _get_module():
    if "nc" not in _CACHE:
        _CACHE["nc"] = _build_module()
    return _CACHE["nc"]


def make_in_maps(x, w_qkv, w_out):
    cos2, sin2 = _rope_tables_np()
    in_maps = []
    for c in range(NCORES):
        b, g = divmod(c, 4)
        q0 = 256 * g
        # column chunks: [q_hp0 | k_hp0 | q_hp1 | k_hp1]
        wqk_c = np.concatenate(
            [
                w_qkv[:, q0 : q0 + 128],
                w_qkv[:, 1024 + q0 : 1024 + q0 + 128],
                w_qkv[:, q0 + 128 : q0 + 256],
                w_qkv[:, 1024 + q0 + 128 : 1024 + q0 + 256],
            ],
            axis=1,
        )
        xt4 = np.ascontiguousarray(
            x[b].T.reshape(8, 128, 4, 512).transpose(2, 1, 0, 3)
        )
        wv_c = w_qkv[:, 2048 + q0 : 2048 + q0 + 256]
        in_maps.append(
            {
                "xT": xt4,
                "w_qk": np.ascontiguousarray(
                    wqk_c.reshape(8, 128, 512).transpose(1, 0, 2)
                ),
                "w_v": np.ascontiguousarray(
                    wv_c.reshape(8, 128, 256).transpose(1, 0, 2)
                ),
                "w_o": np.ascontiguousarray(
                    w_out[q0 : q0 + 256, :].reshape(2, 128, 1024).transpose(1, 0, 2)
                ),
                "cos2": cos2,
                "sin2": sin2,
            }
        )
    return in_maps


def combine_outputs(results, b_out):
    out = np.empty((B, T, D), dtype=np.float32)
    for b in range(B):
        acc = results[4 * b]["out"].astype(np.float32).copy()
        for c in range(4 * b + 1, 4 * b + 4):
            acc += results[c]["out"]
        out[b] = acc + b_out[None, :]
    return out


def kernel(x, w_qkv, w_out, b_out, _trace=False, _tag=[0]):
    from concourse import bass_utils

    nc = _get_module()
    in_maps = make_in_maps(
        np.asarray(x, dtype=np.float32),
        np.asarray(w_qkv, dtype=np.float32),
        np.asarray(w_out, dtype=np.float32),
    )
    res = bass_utils.run_bass_kernel_spmd(
        nc, in_maps, core_ids=list(range(NCORES)), trace=_trace
    )
    if _trace:
        _CACHE["last_result"] = res
    return combine_outputs(res.results, np.asarray(b_out, dtype=np.float32))



# revision 6
# speedup vs baseline: 1.1397x; 1.1397x over previous
"""Multi-head self-attention (RoPE, eval-mode) Trainium2 Bass kernel.

Problem: B=2, T=2048, D=1024, H=16, d_head=64, fp32 I/O.

Sharding (8 cores): core c handles batch b=c//4 and the 4 heads
[4g, 4g+4) where g=c%4.  QKV/attention are head-local; the output
projection produces a per-core partial (contraction over this core's
256 head-dims) which the host sums across the 4 cores of each batch
and adds b_out.

Design (v2):
  - All inputs cast to bf16 on the host (halves DMA bytes; matmuls
    stream at the same 1 col/cycle as fp32r but LDWEIGHTS gets FWL and
    DVE element ops get 2x packing).  PSUM stays fp32; per-core output
    partials are written as bf16 and summed in fp32 on the host.
  - The kernel is one big software-pipelined loop over 8 attention
    blocks (2 head-pairs x 4 tq-blocks of 512 queries) x 16 tk tiles.
    Score matmuls feed exp on the scalar engine (the true bottleneck:
    ~134us of ACTIVATE), PV matmuls consume the exp output one tile
    behind.  QKV projection chains, RoPE, V staging and the output
    projection are emitted as filler inside attention slots so they
    hide in PE slack while ACT stays busy.
  - Prefix: a dummy exp preloads the ACT table set and ~10 junk
    matmuls warm the PE clock (HAM) during the input-DMA window, so
    real chains run at 2.4 GHz from the start.
  - PSUM: sc 2x[128,1024] (4 banks) + pv 2x[128,512] (2) + a shared
    2-bank scratch pool for QKV chains / V staging / outproj.
"""

import numpy as np

B, T, D = 2, 2048, 1024
H = 16
DH = 64
NCORES = 8
P = 128

ROW_TILED = False  # True: K=64 row-group score matmuls (2 heads concurrent)

_CACHE = {}


def _rope_tables_np():
    theta = 1.0 / (10000.0 ** (np.arange(0, DH, 2, dtype=np.float32) / DH))
    angles = np.outer(np.arange(T, dtype=np.float32), theta)  # (T, 32)
    angles = np.concatenate([angles, angles], axis=-1)  # (T, DH)
    cos = np.cos(angles).astype(np.float32)
    sin = np.sin(angles).astype(np.float32)
    cosT = np.ascontiguousarray(cos.T)  # (64, T)
    sinT = np.ascontiguousarray(sin.T)
    sinT_signed = np.concatenate([-sinT[0:32], sinT[32:64]], axis=0)
    cos2 = np.tile(cosT, (2, 1))  # (128, T)
    sin2 = np.tile(sinT_signed, (2, 1))
    return cos2, sin2


def _build_module():
    import concourse.mybir as mybir
    import concourse.tile as tile
    from concourse import bacc

    f32 = mybir.dt.float32
    bf16 = mybir.dt.bfloat16

    nc = bacc.Bacc("TRN2", target_bir_lowering=False, debug=False)
    xT = nc.dram_tensor("xT", [4, P, 8, 512], bf16, kind="ExternalInput")
    w_qk = nc.dram_tensor("w_qk", [P, 8, 512], bf16, kind="ExternalInput")
    w_v = nc.dram_tensor("w_v", [P, 8, 256], bf16, kind="ExternalInput")
    w_o = nc.dram_tensor("w_o", [P, 2, 1024], bf16, kind="ExternalInput")
    cos2 = nc.dram_tensor("cos2", [P, T], bf16, kind="ExternalInput")
    sin2 = nc.dram_tensor("sin2", [P, T], bf16, kind="ExternalInput")
    out = nc.dram_tensor("out", [T, D], bf16, kind="ExternalOutput")

    Exp = mybir.ActivationFunctionType.Exp

    with tile.TileContext(nc) as tc:
        with (
            tc.tile_pool(name="persist", bufs=1) as persist,
            tc.tile_pool(name="expp", bufs=3) as epool,
            tc.tile_pool(name="rope", bufs=2) as rpool,
            tc.tile_pool(name="ob", bufs=3) as opool,
            tc.tile_pool(name="norm", bufs=2) as npool,
            tc.tile_pool(name="sc_ps", bufs=2, space="PSUM") as scps,
            tc.tile_pool(name="pv_ps", bufs=1, space="PSUM") as pvps,
            tc.tile_pool(name="scratch_ps", bufs=2, space="PSUM") as sps,
        ):
            # ---- persistent SBUF ----------------------------------------
            wqk_sb = [
                persist.tile([P, 4, 512], bf16, tag=f"wqk{i}", name=f"wqk{i}")
                for i in range(2)
            ]
            wv_sb = persist.tile([P, 8, 256], bf16)
            wo_sb = persist.tile([P, 2, 1024], bf16)
            cos_sb = persist.tile([P, T], bf16)
            sin_sb = persist.tile([P, T], bf16)
            x_sb = [
                [
                    persist.tile([P, 4, 512], bf16, tag=f"x{q}{h}", name=f"x{q}{h}")
                    for h in range(2)
                ]
                for q in range(4)
            ]
            # roped q / k, two heads stacked on partitions
            q_q = [
                [persist.tile([P, 512], bf16, tag=f"q{hp}_{t}", name=f"q{hp}_{t}")
                 for t in range(4)]
                for hp in range(2)
            ]
            kst = [
                [persist.tile([P, 512], bf16, tag=f"k{hp}_{t}", name=f"k{hp}_{t}")
                 for t in range(4)]
                for hp in range(2)
            ]
            # zero-padded per-head k (only used when not ROW_TILED)
            if not ROW_TILED:
                kpad = [
                    [
                        [persist.tile([P, 512], bf16, tag=f"kp{hp}{h}_{t}",
                                      name=f"kp{hp}{h}_{t}") for t in range(4)]
                        for h in range(2)
                    ]
                    for hp in range(2)
                ]
            # per (tk-tile, head): [ones | v] stationary 128x128
            vaug = persist.tile([P, 16, 4, P], bf16)
            attn_q = [
                [persist.tile([P, 512], bf16, tag=f"at{hp}_{b}", name=f"at{hp}_{b}")
                 for b in range(4)]
                for hp in range(2)
            ]
            warm = persist.tile([P, 512], bf16)
            dummy = persist.tile([P, 16], bf16)
            dummy_o = persist.tile([P, 16], bf16)

            # ---- prefix: PE warmup + ACT table preload + DMAs -----------
            nc.vector.memset(warm[:], 0.0)
            for _ in range(10):
                wps = sps.tile([P, 512], f32, tag="ps", name="ps")
                nc.tensor.matmul(
                    wps[:], lhsT=warm[:, 0:P], rhs=warm[:], start=True, stop=True
                )
            nc.vector.memset(dummy[:], 0.0)
            nc.scalar.activation(dummy_o[:], dummy[:], Exp, scale=0.125)

            # input DMAs in priority order, spread across queues
            nc.sync.dma_start(wqk_sb[0][:], w_qk[:, 0:4, :])
            nc.gpsimd.dma_start(wqk_sb[1][:], w_qk[:, 4:8, :])
            nc.scalar.dma_start(cos_sb[:], cos2[:])
            nc.scalar.dma_start(sin_sb[:], sin2[:])
            nc.sync.dma_start(x_sb[0][0][:], xT[0, :, 0:4, :])
            nc.gpsimd.dma_start(x_sb[0][1][:], xT[0, :, 4:8, :])
            nc.scalar.dma_start(wv_sb[:], w_v[:])
            nc.scalar.dma_start(wo_sb[:], w_o[:])
            for q in range(1, 4):
                nc.sync.dma_start(x_sb[q][0][:], xT[q, :, 0:4, :])
                nc.gpsimd.dma_start(x_sb[q][1][:], xT[q, :, 4:8, :])

            nc.vector.memset(vaug[:, :, :, 0:64], 1.0)
            if not ROW_TILED:
                for hp in range(2):
                    for t in range(4):
                        nc.vector.memset(kpad[hp][0][t][64:128, :], 0.0)
                        nc.vector.memset(kpad[hp][1][t][0:64, :], 0.0)

            # ---- building blocks ----------------------------------------
            def fm_chain(cc, t):
                """Feature-major q (cc 0,2) or stacked k (cc 1,3) chain for
                T-quarter t."""
                hp = cc // 2
                dst = (q_q if cc % 2 == 0 else kst)[hp][t]
                ps = sps.tile([P, 512], f32, tag="ps", name="ps")
                for dc in range(8):
                    nc.tensor.matmul(
                        ps[:],
                        lhsT=wqk_sb[dc // 4][:, dc % 4, cc * P:(cc + 1) * P],
                        rhs=x_sb[t][dc // 4][:, dc % 4, :],
                        start=(dc == 0),
                        stop=(dc == 7),
                    )
                nc.vector.tensor_copy(dst[:], ps[:])

            def v_chain(tk):
                t, t4 = tk // 4, tk % 4
                ps = sps.tile([P, 512], f32, tag="ps", name="ps")
                psv = ps[:, 0:256]
                for dc in range(8):
                    nc.tensor.matmul(
                        psv,
                        lhsT=x_sb[t][dc // 4][:, dc % 4, t4 * P:(t4 + 1) * P],
                        rhs=wv_sb[:, dc, :],
                        start=(dc == 0),
                        stop=(dc == 7),
                    )
                nc.vector.tensor_copy(
                    vaug[:, tk, :, 64:128],
                    psv.rearrange("p (h e) -> p h e", e=64),
                )

            def rope(cc, t):
                """RoPE one T-quarter of q (cc 0,2) or stacked k (cc 1,3),
                in place.  rotate_half = 32-partition block swap via
                SBUF->SBUF DMAs; sign folded into the sin table."""
                hp = cc // 2
                base = (q_q if cc % 2 == 0 else kst)[hp][t]
                hs = slice(t * 512, (t + 1) * 512)
                rot = rpool.tile([P, 512], bf16, tag="rot", name="rot")
                for blk in range(4):
                    s = (blk ^ 1) * 32
                    eng = nc.sync if blk % 2 == 0 else nc.gpsimd
                    eng.dma_start(rot[blk * 32:(blk + 1) * 32, :], base[s:s + 32, :])
                t1 = rpool.tile([P, 512], bf16, tag="t1", name="t1")
                nc.vector.tensor_mul(t1[:], base[:], cos_sb[:, hs])
                nc.vector.tensor_mul(rot[:], rot[:], sin_sb[:, hs])
                nc.vector.tensor_add(base[:], t1[:], rot[:])
                if not ROW_TILED and cc % 2 == 1:
                    nc.vector.tensor_copy(kpad[hp][0][t][0:64, :], base[0:64, :])
                    nc.vector.tensor_copy(kpad[hp][1][t][64:128, :], base[64:128, :])

            def outproj_unit(b, tqc):
                row = b * 4 + tqc
                for d2 in range(2):
                    po = sps.tile([P, 512], f32, tag="ps", name="ps")
                    for hp in range(2):
                        nc.tensor.matmul(
                            po[:],
                            lhsT=attn_q[hp][b][:, tqc * P:(tqc + 1) * P],
                            rhs=wo_sb[:, hp, d2 * 512:(d2 + 1) * 512],
                            start=(hp == 0),
                            stop=(hp == 1),
                        )
                    ob = opool.tile([P, 512], bf16, tag="ob", name="ob")
                    nc.vector.tensor_copy(ob[:], po[:])
                    seng = nc.sync if d2 == 0 else nc.gpsimd
                    seng.dma_start(
                        out[row * P:(row + 1) * P, d2 * 512:(d2 + 1) * 512], ob[:]
                    )

            # ---- slot-planned attention loop ----------------------------
            # blocks: bi 0..7 -> (hp=bi//4, tq=bi%4); 16 tk iters each.
            fillers = {}

            def add(bi, i, fn):
                fillers.setdefault((bi, i), []).append(fn)

            for i in range(16):
                add(0, i, (lambda tk: lambda: v_chain(tk))(i))
            add(0, 1, lambda: fm_chain(1, 1))
            add(0, 2, lambda: rope(1, 1))
            add(0, 3, lambda: fm_chain(0, 1))
            add(0, 4, lambda: rope(0, 1))
            add(0, 5, lambda: fm_chain(1, 2))
            add(0, 6, lambda: rope(1, 2))
            add(0, 9, lambda: fm_chain(1, 3))
            add(0, 10, lambda: rope(1, 3))
            add(0, 12, lambda: fm_chain(0, 2))
            add(0, 13, lambda: rope(0, 2))
            # block 1: hp1 k chains; block 2: hp1 q chains + q(0,3)
            for j, t in enumerate(range(4)):
                add(1, 4 * j + 1, (lambda tt: lambda: fm_chain(3, tt))(t))
                add(1, 4 * j + 3, (lambda tt: lambda: rope(3, tt))(t))
            add(1, 14, lambda: fm_chain(0, 3))
            add(1, 15, lambda: rope(0, 3))
            for j, t in enumerate(range(4)):
                add(2, 4 * j + 1, (lambda tt: lambda: fm_chain(2, tt))(t))
                add(2, 4 * j + 3, (lambda tt: lambda: rope(2, tt))(t))
            # outproj for batch-row block b lands in block 5+b (b=3 at tail)
            for b in range(3):
                for tqc in range(4):
                    add(5 + b, 4 * tqc + 3,
                        (lambda bb, tt: lambda: outproj_unit(bb, tt))(b, tqc))

            # prefix chains: k(hp0, qtr0) and q(hp0, tq0) + ropes
            fm_chain(1, 0)
            fm_chain(0, 0)
            rope(1, 0)
            rope(0, 0)

            for bi in range(8):
                hp, tq = bi // 4, bi % 4
                pv = [
                    pvps.tile([P, 512], f32, tag=f"pv{h}", name=f"pv{h}")
                    for h in range(2)
                ]
                exs = []
                for i in range(16):
                    # scores for iter i
                    sc = scps.tile([P, 1024], f32, tag="sc", name="sc")
                    ko = (i % 4) * P
                    for h in range(2):
                        if ROW_TILED:
                            hsl = slice(h * 64, (h + 1) * 64)
                            nc.tensor.matmul(
                                sc[:, h * 512:(h + 1) * 512],
                                lhsT=kst[hp][i // 4][hsl, ko:ko + P],
                                rhs=q_q[hp][tq][hsl, :],
                                start=True,
                                stop=True,
                            )
                        else:
                            nc.tensor.matmul(
                                sc[:, h * 512:(h + 1) * 512],
                                lhsT=kpad[hp][h][i // 4][:, ko:ko + P],
                                rhs=q_q[hp][tq][:],
                                start=True,
                                stop=True,
                            )
                    ex = epool.tile([P, 1024], bf16, tag="e", name="e")
                    nc.scalar.activation(ex[:], sc[:], Exp, scale=0.125)
                    exs.append(ex)
                    # filler work between sc and pv
                    for fn in fillers.get((bi, i), ()):
                        fn()
                    # PV for iter i
                    for h in range(2):
                        nc.tensor.matmul(
                            pv[h][:],
                            lhsT=vaug[:, i, hp * 2 + h, :],
                            rhs=ex[:, h * 512:(h + 1) * 512],
                            start=(i == 0),
                            stop=(i == 15),
                        )
                # normalize: denominators live in pv rows 0:64 (replicated)
                for h in range(2):
                    rc = npool.tile([64, 512], f32, tag="rc", name="rc")
                    nc.vector.reciprocal_approx_fast(rc[:], pv[h][0:64, :])
                    hb = h * 64
                    nc.vector.tensor_mul(
                        attn_q[hp][tq][hb:hb + 64, :], pv[h][64:128, :], rc[:]
                    )
            for tqc in range(4):
                outproj_unit(3, tqc)

    nc.compile()
    return nc


def _get_module():
    if "nc" not in _CACHE:
        _CACHE["nc"] = _build_module()
    return _CACHE["nc"]


def make_in_maps(x, w_qkv, w_out):
    import ml_dtypes

    bf = ml_dtypes.bfloat16
    cos2, sin2 = _rope_tables_np()
    cos2 = cos2.astype(bf)
    sin2 = sin2.astype(bf)
    in_maps = []
    for c in range(NCORES):
        b, g = divmod(c, 4)
        q0 = 256 * g
        # column chunks: [q_hp0 | k_hp0 | q_hp1 | k_hp1]
        wqk_c = np.concatenate(
            [
                w_qkv[:, q0:q0 + 128],
                w_qkv[:, 1024 + q0:1024 + q0 + 128],
                w_qkv[:, q0 + 128:q0 + 256],
                w_qkv[:, 1024 + q0 + 128:1024 + q0 + 256],
            ],
            axis=1,
        )
        xt4 = np.ascontiguousarray(
            x[b].T.reshape(8, 128, 4, 512).transpose(2, 1, 0, 3)
        ).astype(bf)
        wv_c = w_qkv[:, 2048 + q0:2048 + q0 + 256]
        in_maps.append(
            {
                "xT": xt4,
                "w_qk": np.ascontiguousarray(
                    wqk_c.reshape(8, 128, 512).transpose(1, 0, 2)
                ).astype(bf),
                "w_v": np.ascontiguousarray(
                    wv_c.reshape(8, 128, 256).transpose(1, 0, 2)
                ).astype(bf),
                "w_o": np.ascontiguousarray(
                    w_out[q0:q0 + 256, :].reshape(2, 128, 1024).transpose(1, 0, 2)
                ).astype(bf),
                "cos2": cos2,
                "sin2": sin2,
            }
        )
    return in_maps


def combine_outputs(results, b_out):
    out = np.empty((B, T, D), dtype=np.float32)
    for b in range(B):
        acc = results[4 * b]["out"].astype(np.float32)
        for c in range(4 * b + 1, 4 * b + 4):
            acc += results[c]["out"].astype(np.float32)
        out[b] = acc + b_out[None, :]
    return out


def kernel(x, w_qkv, w_out, b_out, _trace=False, _tag=[0]):
    from concourse import bass_utils

    nc = _get_module()
    in_maps = make_in_maps(
        np.asarray(x, dtype=np.float32),
        np.asarray(w_qkv, dtype=np.float32),
        np.asarray(w_out, dtype=np.float32),
    )
    res = bass_utils.run_bass_kernel_spmd(
        nc, in_maps, core_ids=list(range(NCORES)), trace=_trace
    )
    if _trace:
        _CACHE["last_result"] = res
    return combine_outputs(res.results, np.asarray(b_out, dtype=np.float32))


# revision 8
# speedup vs baseline: 1.1552x; 1.0136x over previous
"""Multi-head self-attention (RoPE, eval-mode) Trainium2 Bass kernel.

Problem: B=2, T=2048, D=1024, H=16, d_head=64, fp32 I/O.

Sharding (8 cores): core c handles batch b=c//4 and the 4 heads
[4g, 4g+4) where g=c%4.  QKV/attention are head-local; the output
projection produces a per-core partial (contraction over this core's
256 head-dims) which the host sums across the 4 cores of each batch
and adds b_out.

Design (v2):
  - All inputs bf16 (PSUM accumulation fp32, output partials bf16,
    host-summed in fp32).  DMAs are full 8KB-per-partition lines.
  - One software-pipelined loop over 8 attention blocks (2 head-pairs
    x 4 tq-blocks of 512 queries) x 16 tk tiles: score matmuls feed
    exp on the scalar engine (the bottleneck, ~135us of ACTIVATE), PV
    consumes exp output one tile behind.  QKV chains, RoPE, V staging
    and outproj are filler inside attention slots.
  - Scores run as K=64 row-group matmul pairs (two heads in disjoint
    32-row strips execute concurrently on the PE).
  - Prefix: dummy exp preloads the ACT table set; junk matmuls keep
    HAM at full clock through the DMA window; tail repeats the trick
    so the last outproj doesn't run at half clock.
"""

import numpy as np

B, T, D = 2, 2048, 1024
H = 16
DH = 64
NCORES = 8
P = 128

ROW_TILED = True  # K=64 row-group score matmuls (2 heads concurrent)

_CACHE = {}


def _rope_tables_np():
    theta = 1.0 / (10000.0 ** (np.arange(0, DH, 2, dtype=np.float32) / DH))
    angles = np.outer(np.arange(T, dtype=np.float32), theta)  # (T, 32)
    angles = np.concatenate([angles, angles], axis=-1)  # (T, DH)
    cos = np.cos(angles).astype(np.float32)
    sin = np.sin(angles).astype(np.float32)
    cosT = np.ascontiguousarray(cos.T)  # (64, T)
    sinT = np.ascontiguousarray(sin.T)
    sinT_signed = np.concatenate([-sinT[0:32], sinT[32:64]], axis=0)
    cos2 = np.tile(cosT, (2, 1))  # (128, T)
    sin2 = np.tile(sinT_signed, (2, 1))
    return cos2, sin2


def _build_module():
    import concourse.mybir as mybir
    import concourse.tile as tile
    from concourse import bacc

    f32 = mybir.dt.float32
    bf16 = mybir.dt.bfloat16

    nc = bacc.Bacc("TRN2", target_bir_lowering=False, debug=False)
    xT = nc.dram_tensor("xT", [4, P, 8, 512], bf16, kind="ExternalInput")
    w_qk = nc.dram_tensor("w_qk", [2, P, 4, 512], bf16, kind="ExternalInput")
    trig = nc.dram_tensor("trig", [P, 2, T], bf16, kind="ExternalInput")
    wvo = nc.dram_tensor("wvo", [P, 2, 2048], bf16, kind="ExternalInput")
    out = nc.dram_tensor("out", [T, D], bf16, kind="ExternalOutput")

    Exp = mybir.ActivationFunctionType.Exp

    with tile.TileContext(nc) as tc:
        with (
            tc.tile_pool(name="persist", bufs=1) as persist,
            tc.tile_pool(name="expp", bufs=3) as epool,
            tc.tile_pool(name="rope", bufs=2) as rpool,
            tc.tile_pool(name="ob", bufs=3) as opool,
            tc.tile_pool(name="norm", bufs=2) as npool,
            tc.tile_pool(name="sc_ps", bufs=2, space="PSUM") as scps,
            tc.tile_pool(name="pv_ps", bufs=1, space="PSUM") as pvps,
            tc.tile_pool(name="scratch_ps", bufs=2, space="PSUM") as sps,
        ):
            # ---- persistent SBUF ----------------------------------------
            wqk_sb = [
                persist.tile([P, 4, 512], bf16, tag=f"wqk{i}", name=f"wqk{i}")
                for i in range(2)
            ]
            trig_sb = persist.tile([P, 2, T], bf16)
            cos_sb = trig_sb[:, 0, :]
            sin_sb = trig_sb[:, 1, :]
            wvo_sb = persist.tile([P, 2, 2048], bf16)
            wv_sb = wvo_sb[:, 0, :].rearrange("p (a b) -> p a b", b=256)
            wo_sb = wvo_sb[:, 1, :].rearrange("p (a b) -> p a b", b=1024)
            x_sb = [
                persist.tile([P, 8, 512], bf16, tag=f"x{q}", name=f"x{q}")
                for q in range(4)
            ]
            # roped q / k, two heads stacked on partitions
            q_q = [
                [persist.tile([P, 512], bf16, tag=f"q{hp}_{t}", name=f"q{hp}_{t}")
                 for t in range(4)]
                for hp in range(2)
            ]
            kst = [
                [persist.tile([P, 512], bf16, tag=f"k{hp}_{t}", name=f"k{hp}_{t}")
                 for t in range(4)]
                for hp in range(2)
            ]
            if not ROW_TILED:
                kpad = [
                    [
                        [persist.tile([P, 512], bf16, tag=f"kp{hp}{h}_{t}",
                                      name=f"kp{hp}{h}_{t}") for t in range(4)]
                        for h in range(2)
                    ]
                    for hp in range(2)
                ]
            # per (tk-tile, head): [ones | v] stationary 128x128
            vaug = persist.tile([P, 16, 4, P], bf16)
            attn_q = [
                [persist.tile([P, 512], bf16, tag=f"at{hp}_{b}", name=f"at{hp}_{b}")
                 for b in range(4)]
                for hp in range(2)
            ]
            warm = persist.tile([P, 512], bf16)
            dummy = persist.tile([P, 16], bf16)
            dummy_o = persist.tile([P, 16], bf16)

            # ---- prefix: PE warmup + ACT table preload ------------------
            nc.vector.memset(warm[:], 0.0)

            def warm_mms(n):
                for _ in range(n):
                    wps = sps.tile([P, 512], f32, tag="ps", name="ps")
                    nc.tensor.matmul(
                        wps[:], lhsT=warm[:, 0:P], rhs=warm[:], start=True,
                        stop=True,
                    )

            warm_mms(10)
            nc.vector.memset(dummy[:], 0.0)
            nc.scalar.activation(dummy_o[:], dummy[:], Exp, scale=0.125)

            # input DMAs: first-needed first; rot/x DMAs interleave later
            nc.sync.dma_start(wqk_sb[0][:], w_qk[0])
            nc.gpsimd.dma_start(wqk_sb[1][:], w_qk[1])
            nc.scalar.dma_start(trig_sb[:], trig[:])
            nc.gpsimd.dma_start(x_sb[0][:], xT[0])
            nc.scalar.dma_start(wvo_sb[:], wvo[:])
            nc.gpsimd.memset(vaug[:, :, :, 0:64], 1.0)
            if not ROW_TILED:
                for hp in range(2):
                    for t in range(4):
                        nc.gpsimd.memset(kpad[hp][0][t][64:128, :], 0.0)
                        nc.gpsimd.memset(kpad[hp][1][t][0:64, :], 0.0)

            # ---- building blocks ----------------------------------------
            def fm_chain(cc, t):
                """Feature-major q (cc 0,2) or stacked k (cc 1,3) chain for
                T-quarter t."""
                hp = cc // 2
                dst = (q_q if cc % 2 == 0 else kst)[hp][t]
                ps = sps.tile([P, 512], f32, tag="ps", name="ps")
                for dc in range(8):
                    nc.tensor.matmul(
                        ps[:],
                        lhsT=wqk_sb[dc // 4][:, dc % 4, cc * P:(cc + 1) * P],
                        rhs=x_sb[t][:, dc, :],
                        start=(dc == 0),
                        stop=(dc == 7),
                    )
                nc.vector.tensor_copy(dst[:], ps[:])

            def v_chain(tk):
                t, t4 = tk // 4, tk % 4
                ps = sps.tile([P, 512], f32, tag="ps", name="ps")
                psv = ps[:, 0:256]
                for dc in range(8):
                    nc.tensor.matmul(
                        psv,
                        lhsT=x_sb[t][:, dc, t4 * P:(t4 + 1) * P],
                        rhs=wv_sb[:, dc, :],
                        start=(dc == 0),
                        stop=(dc == 7),
                    )
                nc.vector.tensor_copy(
                    vaug[:, tk, :, 64:128],
                    psv.rearrange("p (h e) -> p h e", e=64),
                )

            def rope(cc, t):
                """RoPE one T-quarter of q (cc 0,2) or stacked k (cc 1,3),
                in place.  rotate_half = 32-partition block swap via two
                partition-strided SBUF->SBUF DMAs; sign folded into sin."""
                hp = cc // 2
                base = (q_q if cc % 2 == 0 else kst)[hp][t]
                hs = slice(t * 512, (t + 1) * 512)
                rot = rpool.tile([P, 512], bf16, tag="rot", name="rot")
                for blk in range(4):
                    s = (blk ^ 1) * 32
                    eng = nc.sync if blk % 2 == 0 else nc.gpsimd
                    eng.dma_start(rot[blk * 32:(blk + 1) * 32, :], base[s:s + 32, :])
                t1 = rpool.tile([P, 512], bf16, tag="t1", name="t1")
                nc.vector.tensor_mul(t1[:], base[:], cos_sb[:, hs])
                nc.vector.tensor_mul(rot[:], rot[:], sin_sb[:, hs])
                nc.vector.tensor_add(base[:], t1[:], rot[:])
                if not ROW_TILED and cc % 2 == 1:
                    nc.vector.tensor_copy(kpad[hp][0][t][0:64, :], base[0:64, :])
                    nc.vector.tensor_copy(kpad[hp][1][t][64:128, :], base[64:128, :])

            def outproj_unit(b, tqc):
                row = b * 4 + tqc
                for d2 in range(2):
                    po = sps.tile([P, 512], f32, tag="ps", name="ps")
                    for hp in range(2):
                        nc.tensor.matmul(
                            po[:],
                            lhsT=attn_q[hp][b][:, tqc * P:(tqc + 1) * P],
                            rhs=wo_sb[:, hp, d2 * 512:(d2 + 1) * 512],
                            start=(hp == 0),
                            stop=(hp == 1),
                        )
                    ob = opool.tile([P, 512], bf16, tag="ob", name="ob")
                    nc.vector.tensor_copy(ob[:], po[:])
                    seng = nc.sync if d2 == 0 else nc.gpsimd
                    seng.dma_start(
                        out[row * P:(row + 1) * P, d2 * 512:(d2 + 1) * 512], ob[:]
                    )

            # ---- prefix chains ------------------------------------------
            fm_chain(1, 0)
            fm_chain(0, 0)
            rope(1, 0)
            rope(0, 0)
            # remaining x quarters (after the first ropes' rot DMAs so the
            # rot transfers aren't stuck behind megabytes of queue traffic)
            nc.sync.dma_start(x_sb[1][:], xT[1])
            nc.gpsimd.dma_start(x_sb[2][:], xT[2])
            nc.sync.dma_start(x_sb[3][:], xT[3])

            # ---- slot-planned attention loop ----------------------------
            fillers = {}

            def add(bi, i, fn):
                fillers.setdefault((bi, i), []).append(fn)

            for i in range(16):
                add(0, i, (lambda tk: lambda: v_chain(tk))(i))
            add(0, 1, lambda: fm_chain(1, 1))
            add(0, 2, lambda: rope(1, 1))
            add(0, 3, lambda: fm_chain(0, 1))
            add(0, 4, lambda: rope(0, 1))
            add(0, 5, lambda: fm_chain(1, 2))
            add(0, 6, lambda: rope(1, 2))
            add(0, 9, lambda: fm_chain(1, 3))
            add(0, 10, lambda: rope(1, 3))
            add(0, 12, lambda: fm_chain(0, 2))
            add(0, 13, lambda: rope(0, 2))
            for j, t in enumerate(range(4)):
                add(1, 4 * j + 1, (lambda tt: lambda: fm_chain(3, tt))(t))
                add(1, 4 * j + 3, (lambda tt: lambda: rope(3, tt))(t))
            add(1, 14, lambda: fm_chain(0, 3))
            add(1, 15, lambda: rope(0, 3))
            for j, t in enumerate(range(4)):
                add(2, 4 * j + 1, (lambda tt: lambda: fm_chain(2, tt))(t))
                add(2, 4 * j + 3, (lambda tt: lambda: rope(2, tt))(t))
            for b in range(3):
                for tqc in range(4):
                    add(5 + b, 4 * tqc + 3,
                        (lambda bb, tt: lambda: outproj_unit(bb, tt))(b, tqc))

            for bi in range(8):
                hp, tq = bi // 4, bi % 4
                pv = [
                    pvps.tile([P, 512], f32, tag=f"pv{h}", name=f"pv{h}")
                    for h in range(2)
                ]
                for i in range(16):
                    sc = scps.tile([P, 1024], f32, tag="sc", name="sc")
                    ko = (i % 4) * P
                    for h in range(2):
                        if ROW_TILED:
                            hsl = slice(h * 64, (h + 1) * 64)
                            nc.tensor.matmul(
                                sc[:, h * 512:(h + 1) * 512],
                                lhsT=kst[hp][i // 4][hsl, ko:ko + P],
                                rhs=q_q[hp][tq][hsl, :],
                                start=True,
                                stop=True,
                            )
                        else:
                            nc.tensor.matmul(
                                sc[:, h * 512:(h + 1) * 512],
                                lhsT=kpad[hp][h][i // 4][:, ko:ko + P],
                                rhs=q_q[hp][tq][:],
                                start=True,
                                stop=True,
                            )
                    ex = epool.tile([P, 1024], bf16, tag="e", name="e")
                    nc.scalar.activation(ex[:], sc[:], Exp, scale=0.125)
                    for fn in fillers.get((bi, i), ()):
                        fn()
                    for h in range(2):
                        nc.tensor.matmul(
                            pv[h][:],
                            lhsT=vaug[:, i, hp * 2 + h, :],
                            rhs=ex[:, h * 512:(h + 1) * 512],
                            start=(i == 0),
                            stop=(i == 15),
                        )
                if bi == 7:
                    # keep HAM warm through the norm window so the tail
                    # outproj runs at full clock
                    warm_mms(8)
                for h in range(2):
                    rc = npool.tile([64, 512], f32, tag="rc", name="rc")
                    nc.vector.reciprocal_approx_fast(rc[:], pv[h][0:64, :])
                    hb = h * 64
                    nc.vector.tensor_mul(
                        attn_q[hp][tq][hb:hb + 64, :], pv[h][64:128, :], rc[:]
                    )
            for tqc in range(4):
                outproj_unit(3, tqc)

    nc.compile()
    return nc


def _get_module():
    if "nc" not in _CACHE:
        _CACHE["nc"] = _build_module()
    return _CACHE["nc"]


def make_in_maps(x, w_qkv, w_out):
    import ml_dtypes

    bf = ml_dtypes.bfloat16
    cos2, sin2 = _rope_tables_np()
    trig = np.stack([cos2, sin2], axis=1).astype(bf)  # (128, 2, T)
    in_maps = []
    for c in range(NCORES):
        b, g = divmod(c, 4)
        q0 = 256 * g
        # column chunks: [q_hp0 | k_hp0 | q_hp1 | k_hp1]
        wqk_c = np.concatenate(
            [
                w_qkv[:, q0:q0 + 128],
                w_qkv[:, 1024 + q0:1024 + q0 + 128],
                w_qkv[:, q0 + 128:q0 + 256],
                w_qkv[:, 1024 + q0 + 128:1024 + q0 + 256],
            ],
            axis=1,
        )
        wqk8 = np.ascontiguousarray(
            wqk_c.reshape(8, 128, 512).transpose(1, 0, 2)
        )  # (128, 8, 512)
        wqk2 = np.ascontiguousarray(
            wqk8.reshape(128, 2, 4, 512).transpose(1, 0, 2, 3)
        ).astype(bf)  # (2, 128, 4, 512)
        xt4 = np.ascontiguousarray(
            x[b].T.reshape(8, 128, 4, 512).transpose(2, 1, 0, 3)
        ).astype(bf)  # (4, 128, 8, 512)
        wv_c = np.ascontiguousarray(
            w_qkv[:, 2048 + q0:2048 + q0 + 256].reshape(8, 128, 256)
            .transpose(1, 0, 2)
        ).reshape(128, 2048)
        wo_c = np.ascontiguousarray(
            w_out[q0:q0 + 256, :].reshape(2, 128, 1024).transpose(1, 0, 2)
        ).reshape(128, 2048)
        wvo = np.ascontiguousarray(
            np.stack([wv_c, wo_c], axis=1)
        ).astype(bf)  # (128, 2, 2048)
        in_maps.append(
            {"xT": xt4, "w_qk": wqk2, "trig": trig, "wvo": wvo}
        )
    return in_maps


def combine_outputs(results, b_out):
    out = np.empty((B, T, D), dtype=np.float32)
    for b in range(B):
        acc = results[4 * b]["out"].astype(np.float32)
        for c in range(4 * b + 1, 4 * b + 4):
            acc += results[c]["out"].astype(np.float32)
        out[b] = acc + b_out[None, :]
    return out


def kernel(x, w_qkv, w_out, b_out, _trace=False, _tag=[0]):
    from concourse import bass_utils

    nc = _get_module()
    in_maps = make_in_maps(
        np.asarray(x, dtype=np.float32),
        np.asarray(w_qkv, dtype=np.float32),
        np.asarray(w_out, dtype=np.float32),
    )
    res = bass_utils.run_bass_kernel_spmd(
        nc, in_maps, core_ids=list(range(NCORES)), trace=_trace
    )
    if _trace:
        _CACHE["last_result"] = res
    return combine_outputs(res.results, np.asarray(b_out, dtype=np.float32))
